# revision 9
# baseline (speedup 1.0000x reference)
"""MoE routing block (nn_Block_5454608466019) on 8 Trainium2 NeuronCores.

Strategy (expert parallelism + AllToAll router exchange):
  - Core e owns expert e (its ew1/ew2 shard) and token group e (data-parallel
    shared expert + router shard).
  - On-device per core: fp32 router matmul over OWN group's 2048 tokens for
    all 8 experts -> softmax -> PE-transpose to expert-major -> AllToAll so
    core e receives expert-e probs for all 16384 tokens -> bisection for the
    capacity-th largest prob -> masked-iota + sparse_gather compaction ->
    transpose-mode dma_gather of the selected token rows (directly d-major)
    -> expert FFN in bf16 with single-pass weight streaming and a resident
    2048-token hidden tile -> write y^T; plus the shared-expert FFN over its
    own group.
  - Host: shard prep, then combine: out[idx_e] += y_e * probs[idx_e, e],
    plus the per-group shared outputs.

kernel(**inputs) takes the full unsharded inputs and returns the full
(G, S, D) float32 output.
"""

import os
from contextlib import ExitStack

import numpy as np

G, S, D, H, E = 8, 2048, 1024, 4096, 8
T = G * S
CAP = 2048
N_BISECT = 36
IDX_W = 136
SGW = 132  # sparse_gather per-half compaction list width (>128 guards over-count)

KD = D // 128   # 8 contraction tiles for FFN1 / router
MH = H // 128   # 32 h-tiles
ND = D // 128   # 8 output d-tiles

_CACHE = {}


def _build():
    import concourse.bacc as bacc
    import concourse.mybir as mybir
    import concourse.tile as tile
    from concourse.masks import make_identity

    F32 = mybir.dt.float32
    BF16 = mybir.dt.bfloat16

    nc = bacc.Bacc("TRN2", target_bir_lowering=False, debug=False)

    # ---------------- I/O ----------------
    xtg32 = nc.dram_tensor("xtg32", [D, S], F32, kind="ExternalInput").ap()
    xgbf = nc.dram_tensor("xgbf", [D, S], BF16, kind="ExternalInput").ap()
    xnat = nc.dram_tensor("xnat", [T, D], BF16, kind="ExternalInput").ap()
    gate = nc.dram_tensor("gate", [D, E], F32, kind="ExternalInput").ap()
    iota = nc.dram_tensor("iota_p1", [16, 1024], F32, kind="ExternalInput").ap()
    sio = nc.dram_tensor("sio", [16, SGW], F32, kind="ExternalInput").ap()
    ew1 = nc.dram_tensor("ew1", [D, H], BF16, kind="ExternalInput").ap()
    eb1 = nc.dram_tensor("eb1", [H], F32, kind="ExternalInput").ap()
    ew2 = nc.dram_tensor("ew2", [H, D], BF16, kind="ExternalInput").ap()
    eb2 = nc.dram_tensor("eb2", [D], F32, kind="ExternalInput").ap()
    sw1 = nc.dram_tensor("sw1", [D, H], BF16, kind="ExternalInput").ap()
    sb1 = nc.dram_tensor("sb1", [H], F32, kind="ExternalInput").ap()
    sw2 = nc.dram_tensor("sw2", [H, D], BF16, kind="ExternalInput").ap()
    sb2 = nc.dram_tensor("sb2", [D], F32, kind="ExternalInput").ap()

    yt_o = nc.dram_tensor("yt", [D, CAP], F32, kind="ExternalOutput").ap()
    sht_o = nc.dram_tensor("sht", [D, S], F32, kind="ExternalOutput").ap()
    idx_o = nc.dram_tensor("idxo", [16, IDX_W], F32, kind="ExternalOutput").ap()

    AX = mybir.AxisListType
    OP = mybir.AluOpType
    AF = mybir.ActivationFunctionType
    # CoreSim has no Gelu; use Tanh there (same cost) for timing/structure
    ACT1 = AF.Tanh if os.environ.get("MOE_SIM") else AF.Gelu_apprx_tanh

    with tile.TileContext(nc) as tc, ExitStack() as ctx:
        const = ctx.enter_context(tc.tile_pool(name="const", bufs=1))
        bis = ctx.enter_context(tc.tile_pool(name="bis", bufs=1))
        dram = ctx.enter_context(tc.tile_pool(name="dram", bufs=1, space="DRAM"))

        # ---------------- constants ----------------
        gate_sb = const.tile([128, KD, E], F32, tag="gate")
        nc.sync.dma_start(gate_sb, gate.rearrange("(kd p) e -> p kd e", p=128))
        iota_sb = const.tile([16, 1024], F32, tag="iota")
        nc.sync.dma_start(iota_sb, iota)
        eb1_sb = const.tile([128, MH], F32, tag="eb1")
        nc.sync.dma_start(eb1_sb, eb1.rearrange("(m p) -> p m", p=128))
        eb2_sb = const.tile([128, ND], F32, tag="eb2")
        nc.sync.dma_start(eb2_sb, eb2.rearrange("(n p) -> p n", p=128))
        sb1_sb = const.tile([128, MH], F32, tag="sb1")
        nc.sync.dma_start(sb1_sb, sb1.rearrange("(m p) -> p m", p=128))
        sb2_sb = const.tile([128, ND], F32, tag="sb2")
        nc.sync.dma_start(sb2_sb, sb2.rearrange("(n p) -> p n", p=128))
        ident = const.tile([128, 128], F32, tag="ident")
        make_identity(nc, ident)

        # ---------------- phase 1: router over own group ----------------
        # probs_tl[l, e, h] = softmax prob of local token s = h*128+l, expert e
        probs_tl = bis.tile([128, E, 16], F32, tag="probs_tl")
        with ExitStack() as rctx:
            router = rctx.enter_context(tc.tile_pool(name="router", bufs=2))
            psum_r = rctx.enter_context(
                tc.tile_pool(name="psum_r", bufs=2, space="PSUM"))
            for h in range(16):
                xt_t = router.tile([128, KD, 128], F32, tag="xt_t")
                nc.sync.dma_start(
                    xt_t,
                    xtg32[:, h * 128:(h + 1) * 128].rearrange(
                        "(kd p) t -> p kd t", p=128))
                ps = psum_r.tile([128, E], F32, tag="ps_r")
                for kd in range(KD):
                    nc.tensor.matmul(ps, lhsT=xt_t[:, kd, :],
                                     rhs=gate_sb[:, kd, :],
                                     start=(kd == 0), stop=(kd == KD - 1))
                nmx = router.tile([128, 1], F32, tag="nmx")
                nc.vector.tensor_reduce(nmx, ps, axis=AX.X, op=OP.max,
                                        negate=True)
                ex = router.tile([128, E], F32, tag="ex")
                nc.scalar.activation(ex, ps, AF.Exp, bias=nmx, scale=1.0)
                sm = router.tile([128, 1], F32, tag="sm")
                nc.vector.tensor_reduce(sm, ex, axis=AX.X, op=OP.add)
                rc = router.tile([128, 1], F32, tag="rc")
                nc.vector.reciprocal(rc, sm)
                nc.vector.tensor_scalar_mul(probs_tl[:, :, h], ex, rc)

            # transpose [l, (e h)] -> [(e h), l]; flat = e*2048 + h*128 + l
            pt_ps = psum_r.tile([128, 128], F32, tag="pt_ps")
            nc.tensor.transpose(pt_ps, probs_tl.rearrange("p e h -> p (e h)"),
                                ident)
            pe_sb = router.tile([128, 128], F32, tag="pe_sb")
            nc.vector.tensor_copy(pe_sb, pt_ps)

            pout_d = dram.tile([128, 128], F32, tag="pout")
            nc.sync.dma_start(pout_d, pe_sb)
            pin_d = dram.tile([128, 128], F32, tag="pin")
            nc.gpsimd.collective_compute(
                "AllToAll",
                mybir.AluOpType.bypass,
                replica_groups=[list(range(8))],
                ins=[pout_d.opt()],
                outs=[pin_d.opt()],
            )
            # prob_all[p, f] = prob of global token t = p*128 + f (own expert)
            prob_all = bis.tile([128, 128], F32, tag="prob_all")
            nc.sync.dma_start(prob_all, pin_d)

        # ---------------- phase 2: bisection for theta ----------------
        lo = bis.tile([1, 1], F32, tag="lo")
        hi = bis.tile([1, 1], F32, tag="hi")
        mid = bis.tile([1, 1], F32, tag="mid")
        total = bis.tile([1, 1], F32, tag="total")
        geq = bis.tile([1, 1], mybir.dt.uint32, tag="geq")
        ltq = bis.tile([1, 1], mybir.dt.uint32, tag="ltq")
        mid128 = bis.tile([128, 32], F32, tag="mid128")
        cmp = bis.tile([128, 128], F32, tag="cmp")
        partial = bis.tile([128, 32], F32, tag="partial")
        tp = bis.tile([32, 128], F32, tag="tp")
        ones32 = bis.tile([1, 32], F32, tag="ones32")
        rep32 = bis.tile([32, 32], F32, tag="rep32")
        th_pad = bis.tile([32, 32], F32, tag="th_pad")

        nc.vector.memset(lo, 0.0)
        nc.vector.memset(hi, 1.0)
        nc.vector.memset(mid, 0.5)
        nc.vector.memset(mid128, 0.5)
        nc.vector.memset(partial, 0.0)
        nc.vector.memset(ones32, 1.0)
        nc.vector.memset(rep32, 0.0)

        for it in range(N_BISECT):
            nc.vector.tensor_scalar(cmp, prob_all, mid128[:, 0:1], 0.0,
                                    op0=OP.is_ge, op1=OP.add,
                                    accum_out=partial[:, 0:1])
            for r in range(4):
                nc.vector.transpose(tp[:, r * 32:(r + 1) * 32],
                                    partial[r * 32:(r + 1) * 32, :])
            nc.vector.tensor_reduce(total, tp[0:1, :], axis=AX.X, op=OP.add)
            nc.vector.tensor_scalar(geq, total, float(CAP), None, op0=OP.is_ge)
            nc.vector.tensor_scalar(ltq, total, float(CAP), None, op0=OP.is_lt)
            nc.vector.copy_predicated(lo, geq, mid)
            nc.vector.copy_predicated(hi, ltq, mid)
            nc.vector.tensor_add(mid, lo, hi)
            nc.vector.tensor_scalar_mul(mid, mid, 0.5)
            if it < N_BISECT - 1:
                nc.vector.tensor_scalar_mul(rep32[0:1, :], ones32, mid)
                for r in range(4):
                    nc.vector.transpose(mid128[r * 32:(r + 1) * 32, :], rep32)

        nc.vector.tensor_scalar_mul(rep32[0:1, :], ones32, lo)
        nc.vector.transpose(th_pad, rep32)

        # ---------------- phase 3: compaction + indices ----------------
        # prob16[r, c] = prob of token r*1024 + c (partition-major reshape)
        prob16 = bis.tile([16, 1024], F32, tag="prob16")
        nc.sync.dma_start(prob16, prob_all)
        cmp16 = bis.tile([16, 1024], F32, tag="cmp16")
        nc.vector.tensor_scalar(cmp16, prob16, th_pad[0:16, 0:1], None,
                                op0=OP.is_ge)
        miota = bis.tile([16, 1024], F32, tag="miota")
        nc.vector.tensor_mul(miota, cmp16, iota_sb)
        nc.vector.tensor_scalar(miota, miota, 1.0, None, op0=OP.subtract)

        # HW sparse_gather input is limited to [16, 512] and leaves garbage
        # beyond num_found in its output.  So: compact each 512-wide half,
        # mask each half-list to its num_found (slot_iota < nf), then merge
        # the two -1-padded lists with a final compaction.  The bisection
        # invariant (count >= CAP) makes the first CAP merged slots valid.
        HW_ = SGW
        halves = bis.tile([16, 2 * HW_], F32, tag="halves")
        nfa = bis.tile([1, 1], mybir.dt.uint32, tag="nfa")
        nfb = bis.tile([1, 1], mybir.dt.uint32, tag="nfb")
        nc.gpsimd.sparse_gather(halves[:, :HW_], miota[:, :512], num_found=nfa)
        nc.gpsimd.sparse_gather(halves[:, HW_:], miota[:, 512:], num_found=nfb)

        sio_sb = const.tile([16, HW_], F32, tag="sio")
        nc.sync.dma_start(sio_sb, sio)
        clean = bis.tile([16, 2 * HW_], F32, tag="clean")
        nc.vector.memset(clean, -1.0)
        for half, nf_t in ((0, nfa), (1, nfb)):
            nf_f = bis.tile([1, 1], F32, tag=f"nf_f{half}")
            nc.vector.tensor_copy(nf_f, nf_t)
            nf_pad = bis.tile([32, 32], F32, tag=f"nf_pad{half}")
            nc.vector.tensor_scalar_mul(rep32[0:1, :], ones32, nf_f)
            nc.vector.transpose(nf_pad, rep32)
            msk = bis.tile([16, HW_], mybir.dt.uint8, tag=f"msk{half}")
            nc.vector.tensor_scalar(msk, sio_sb, nf_pad[0:16, 0:1], None,
                                    op0=OP.is_lt)
            nc.vector.copy_predicated(
                clean[:, half * HW_:(half + 1) * HW_], msk,
                halves[:, half * HW_:(half + 1) * HW_])

        idx_f = bis.tile([16, IDX_W], F32, tag="idx_f")
        nf1 = bis.tile([1, 1], mybir.dt.uint32, tag="nf1")
        nc.gpsimd.sparse_gather(idx_f, clean, num_found=nf1)
        nc.sync.dma_start(idx_o, idx_f)

        idx_i = bis.tile([16, 128], mybir.dt.int16, tag="idx_i")
        nc.vector.tensor_copy(idx_i, idx_f[:, :128])
        idx_rep = bis.tile([128, 128], mybir.dt.int16, tag="idx_rep")
        for k in range(8):
            nc.sync.dma_start(idx_rep[k * 16:(k + 1) * 16, :], idx_i)

        # ---------------- FFN emitter (single weight pass) ----------------
        hpool = ctx.enter_context(tc.tile_pool(name="hpool", bufs=1))
        w1pool = ctx.enter_context(tc.tile_pool(name="w1pool", bufs=3))
        w2pool = ctx.enter_context(tc.tile_pool(name="w2pool", bufs=2))
        ypool = ctx.enter_context(tc.tile_pool(name="ypool", bufs=2))
        rhspool = ctx.enter_context(tc.tile_pool(name="rhspool", bufs=1))
        psum_f = ctx.enter_context(tc.tile_pool(name="psum_f", bufs=2,
                                                space="PSUM"))
        psum2 = ctx.enter_context(tc.tile_pool(name="psum2", bufs=1,
                                               space="PSUM"))

        def emit_ffn(rhs_at, w1, b1_sb, w2, b2_sb, out_dram):
            # rhs_at(kd, c): [128, 512] BF16 d-major token chunk
            hT = hpool.tile([128, MH, S], BF16, tag="hT")
            for m in range(MH):
                w1_t = w1pool.tile([128, KD, 128], BF16, tag="w1")
                nc.sync.dma_start(
                    w1_t,
                    w1[:, m * 128:(m + 1) * 128].rearrange(
                        "(kd p) h -> p kd h", p=128))
                for c in range(S // 512):
                    ph = psum_f.tile([128, 512], F32, tag="ffn")
                    for kd in range(KD):
                        nc.tensor.matmul(ph, lhsT=w1_t[:, kd, :],
                                         rhs=rhs_at(kd, c),
                                         start=(kd == 0), stop=(kd == KD - 1))
                    nc.scalar.activation(hT[:, m, c * 512:(c + 1) * 512], ph,
                                         ACT1,
                                         bias=b1_sb[:, m:m + 1], scale=1.0)
            for n in range(ND):
                # 4 accumulators live across both w2 half-tile loads
                pys = [psum2.tile([128, 512], F32, tag=f"py{c}",
                                  name=f"py{c}")
                       for c in range(S // 512)]
                for khh in range(2):  # stream w2 in half-tiles to save SBUF
                    w2_t = w2pool.tile([128, MH // 2, 128], BF16, tag="w2")
                    nc.sync.dma_start(
                        w2_t,
                        w2[khh * (H // 2):(khh + 1) * (H // 2),
                           n * 128:(n + 1) * 128].rearrange(
                               "(kh p) d -> p kh d", p=128))
                    for c in range(S // 512):
                        for k2 in range(MH // 2):
                            kh = khh * (MH // 2) + k2
                            nc.tensor.matmul(
                                pys[c], lhsT=w2_t[:, k2, :],
                                rhs=hT[:, kh, c * 512:(c + 1) * 512],
                                start=(kh == 0), stop=(kh == MH - 1))
                for c in range(S // 512):
                    yb = ypool.tile([128, 512], F32, tag="yb")
                    nc.scalar.activation(yb, pys[c], AF.Identity,
                                         bias=b2_sb[:, n:n + 1], scale=1.0)
                    nc.sync.dma_start(
                        out_dram.rearrange("(n p) t -> p n t", p=128)
                        [:, n, c * 512:(c + 1) * 512], yb)

        # ---------------- phase 4: shared expert over own group ----------
        # rhs chunk tiles are shared (same tags) with the routed phase; the
        # Tile WAR dependency delays each gather until shared FFN1's last
        # read of that chunk, which overlaps shared FFN2's matmuls.
        sh_rhs = []
        for c in range(S // 512):
            shc = rhspool.tile([128, KD, 512], BF16, tag=f"rhs{c}",
                               name=f"shrhs{c}")
            nc.sync.dma_start(
                shc, xgbf[:, c * 512:(c + 1) * 512].rearrange(
                    "(kd p) t -> p kd t", p=128))
            sh_rhs.append(shc)
        emit_ffn(lambda kd, c: sh_rhs[c][:, kd, :],
                 sw1, sb1_sb, sw2, sb2_sb, sht_o)

        # ---------------- phase 5: routed expert ----------------
        rts = []
        for c in range(S // 512):
            rtc = rhspool.tile([128, KD, 512], BF16, tag=f"rhs{c}",
                               name=f"rt{c}")
            nc.gpsimd.dma_gather(rtc, xnat, idx_rep[:, c * 32:(c + 1) * 32],
                                 num_idxs=512, num_idxs_reg=512,
                                 elem_size=D, transpose=True)
            rts.append(rtc)
        emit_ffn(lambda kd, c: rts[c][:, kd, :],
                 ew1, eb1_sb, ew2, eb2_sb, yt_o)

    nc.compile()
    return nc


def _get_nc():
    if "nc" not in _CACHE:
        _CACHE["nc"] = _build()
    return _CACHE["nc"]


def _prepare_in_maps(inputs):
    import ml_dtypes

    bf16 = ml_dtypes.bfloat16

    x = np.asarray(inputs["x"], dtype=np.float32).reshape(T, D)
    gate_w = np.asarray(inputs["gate_w"], dtype=np.float32)
    ew1 = np.asarray(inputs["ew1"], dtype=np.float32)
    eb1 = np.asarray(inputs["eb1"], dtype=np.float32)
    ew2 = np.asarray(inputs["ew2"], dtype=np.float32)
    eb2 = np.asarray(inputs["eb2"], dtype=np.float32)
    sw1 = np.asarray(inputs["sw1"], dtype=np.float32)
    sb1 = np.asarray(inputs["sb1"], dtype=np.float32)
    sw2 = np.asarray(inputs["sw2"], dtype=np.float32)
    sb2 = np.asarray(inputs["sb2"], dtype=np.float32)
    assert int(inputs["expert_capacity"]) == CAP

    xt = np.ascontiguousarray(x.T)                     # [D, T] f32
    xnat = np.ascontiguousarray(x.astype(bf16))        # [T, D] bf16

    iota_p1 = (np.arange(T) + 1).astype(np.float32).reshape(16, 1024)
    js = np.arange(16 * SGW)
    sio = ((js % SGW) * 16 + js // SGW).astype(np.float32).reshape(16, SGW)

    sw1b = np.ascontiguousarray(sw1.astype(bf16))
    sw2b = np.ascontiguousarray(sw2.astype(bf16))

    in_maps = []
    for e in range(E):
        xg = np.ascontiguousarray(xt[:, e * S:(e + 1) * S])
        in_maps.append({
            "xtg32": xg,
            "xgbf": xg.astype(bf16),
            "xnat": xnat,
            "gate": gate_w,
            "iota_p1": iota_p1,
            "sio": sio,
            "ew1": np.ascontiguousarray(ew1[e].astype(bf16)),
            "eb1": eb1[e],
            "ew2": np.ascontiguousarray(ew2[e].astype(bf16)),
            "eb2": eb2[e],
            "sw1": sw1b,
            "sb1": sb1,
            "sw2": sw2b,
            "sb2": sb2,
        })
    return in_maps, x, gate_w


def _combine(x, gate_w, results):
    logits = x @ gate_w
    m = logits.max(axis=1, keepdims=True)
    ex = np.exp(logits - m)
    probs = ex / ex.sum(axis=1, keepdims=True)

    out = np.zeros((T, D), dtype=np.float32)
    for e in range(E):
        r = results[e]
        out[e * S:(e + 1) * S] += r["sht"].T
        idx = r["idxo"][:, :128].T.ravel().astype(np.int64)
        valid = (idx >= 0) & (idx < T)
        iv = idx[valid]
        out[iv] += r["yt"].T[valid] * probs[iv, e][:, None]
    return out.reshape(G, S, D)


def kernel(**inputs):
    from concourse.bass_utils import run_bass_kernel_spmd

    nc = _get_nc()
    in_maps, x, gate_w = _prepare_in_maps(inputs)

    trace = os.environ.get("MOE_TRACE", "0") == "1"
    res = run_bass_kernel_spmd(nc, in_maps, core_ids=list(range(E)),
                               trace=trace)
    global LAST_EXEC_NS
    LAST_EXEC_NS = res.exec_time_ns
    if res.exec_time_ns is not None:
        print(f"[kernel] HW exec time: {res.exec_time_ns} ns "
              f"(mean {res.mean_exec_time_ns})")

    return _combine(x, gate_w, res.results)
